# revision 1
# baseline (speedup 1.0000x reference)
"""Multi-head attention (B=2, S=2048, D=1024, H=16, d_head=64) on 8 NeuronCores.

Sharding: tensor-parallel over heads. Core c owns heads {2c, 2c+1} for both
batches (4 (head,batch) pairs per core, pair index = 2*hh + b). Each core:
  - projects Q^T/K^T (fp32r) for its heads from replicated x^T,
  - computes full raw scores [q,k] (DMA'd out, pre-mask, already /sqrt(64)
    because Wq/bq are pre-scaled on host),
  - computes causal softmax @ V entirely in transposed [k,q] orientation,
    with the softmax denominator obtained free via a ones-column fused into
    the V stationary operand,
  - computes its partial output projection (row-slice of Wo); partials are
    summed on the host.
Outputs per core: raw4 [4, S, S] (raw score shard) and pout [B*S, D] partial.
"""
import os
import sys

sys.path.insert(0, "/opt/trn_rl_repo")

import numpy as np

import concourse.bacc as bacc
import concourse.mybir as mybir
from concourse.tile import TileContext
from concourse.bass_utils import run_bass_kernel_spmd

P = 128
B, S, D, H, DH = 2, 2048, 1024, 16, 64
T = B * S                  # 4096 tokens
NDT = D // P               # 8 d-model tiles
NSP = T // 512             # 8 token spans (4 per batch)
NKT = S // P               # 16 key tiles per batch
NQS = S // 512             # 4 q spans per batch
F32 = mybir.dt.float32
F32R = mybir.dt.float32r
Copy = mybir.ActivationFunctionType.Copy
Identity = mybir.ActivationFunctionType.Identity
Exp = mybir.ActivationFunctionType.Exp

_COMPILED = None


def _build():
    nc = bacc.Bacc("TRN2", target_bir_lowering=False, debug=False, num_devices=8)

    xT = nc.declare_dram_parameter("xT", [D, T], F32, isOutput=False)
    wq = nc.declare_dram_parameter("wq", [D, P], F32, isOutput=False)
    wk = nc.declare_dram_parameter("wk", [D, P], F32, isOutput=False)
    wv = nc.declare_dram_parameter("wv", [D, P], F32, isOutput=False)
    wo = nc.declare_dram_parameter("wo", [P, D], F32, isOutput=False)
    bq = nc.declare_dram_parameter("bq", [P, 1], F32, isOutput=False)
    bk = nc.declare_dram_parameter("bk", [P, 1], F32, isOutput=False)
    bv = nc.declare_dram_parameter("bv", [P, 1], F32, isOutput=False)
    mneg = nc.declare_dram_parameter("mneg", [P, 4, 512], F32, isOutput=False)
    ones = nc.declare_dram_parameter("ones", [P, 2, B, NKT, 1], F32, isOutput=False)
    ident = nc.declare_dram_parameter("ident", [P, P], F32, isOutput=False)

    raw4 = nc.declare_dram_parameter("raw4", [4, S, S], F32, isOutput=True)
    pout = nc.declare_dram_parameter("pout", [T, D], F32, isOutput=True)

    with TileContext(nc) as tc:
        with tc.tile_pool(name="persist", bufs=1) as persist, \
             tc.tile_pool(name="consts", bufs=1) as consts:
            qt_sb = persist.tile([P, T], F32R, tag="qt")       # Q^T (2 heads x 64, tokens)
            kt_sb = persist.tile([P, T], F32R, tag="kt")       # K^T
            ctxT = persist.tile([P, T], F32R, tag="ctxT")      # Ctx^T
            vaug = persist.tile([P, 2, B, NKT, DH + 1], F32R, tag="vaug")
            wq_sb = consts.tile([P, NDT, P], F32R, tag="wq")
            wk_sb = consts.tile([P, NDT, P], F32R, tag="wk")
            wv_sb = consts.tile([P, NDT, P], F32R, tag="wv")
            wo_sb = consts.tile([P, D], F32R, tag="wo")
            bq_sb = consts.tile([P, 1], F32, tag="bq")
            bk_sb = consts.tile([P, 1], F32, tag="bk")
            bv_sb = consts.tile([P, 1], F32, tag="bv")
            mneg_sb = consts.tile([P, 4, 512], F32, tag="mneg")
            id_sb = consts.tile([P, P], F32, tag="ident")

            for w_dram, w_tile in ((wq, wq_sb), (wk, wk_sb), (wv, wv_sb)):
                nc.gpsimd.dma_start(
                    out=w_tile[:], in_=w_dram[:].rearrange("(dt p) m -> p dt m", p=P))
            nc.gpsimd.dma_start(out=wo_sb[:], in_=wo[:])
            nc.gpsimd.dma_start(out=vaug[:, :, :, :, DH:DH + 1], in_=ones[:])
            nc.sync.dma_start(out=bq_sb[:], in_=bq[:])
            nc.sync.dma_start(out=bk_sb[:], in_=bk[:])
            nc.sync.dma_start(out=bv_sb[:], in_=bv[:])
            nc.sync.dma_start(out=mneg_sb[:], in_=mneg[:])
            nc.sync.dma_start(out=id_sb[:], in_=ident[:])

            xT_v = xT[:].rearrange("(dt p) t -> p dt t", p=P)

            # ---------------- Phase 1: projections ----------------
            with tc.tile_pool(name="xsp", bufs=2) as xsp, \
                 tc.tile_pool(name="vstage", bufs=3) as vstage, \
                 tc.tile_pool(name="proj_ps", bufs=3, space="PSUM") as proj_ps, \
                 tc.tile_pool(name="vt_ps", bufs=2, space="PSUM") as vt_ps:
                for sp in range(NSP):
                    t0 = sp * 512
                    xs = xsp.tile([P, NDT, 512], F32R, tag="xs")
                    nc.gpsimd.dma_start(out=xs[:], in_=xT_v[:, :, t0:t0 + 512])
                    for w_tile, b_tile, dst in (
                            (wq_sb, bq_sb, qt_sb), (wk_sb, bk_sb, kt_sb)):
                        ps = proj_ps.tile([P, 512], F32, tag="proj")
                        for dt_i in range(NDT):
                            nc.tensor.matmul(ps[:], w_tile[:, dt_i, :], xs[:, dt_i, :],
                                             start=(dt_i == 0), stop=(dt_i == NDT - 1))
                        nc.scalar.activation(dst[:, t0:t0 + 512], ps[:], Identity,
                                             bias=b_tile[:])
                    # V^T for this span, then transpose to V tiles
                    ps = proj_ps.tile([P, 512], F32, tag="proj")
                    for dt_i in range(NDT):
                        nc.tensor.matmul(ps[:], wv_sb[:, dt_i, :], xs[:, dt_i, :],
                                         start=(dt_i == 0), stop=(dt_i == NDT - 1))
                    vt_stage = vstage.tile([P, 512], F32, tag="vt")
                    nc.scalar.activation(vt_stage[:], ps[:], Identity, bias=bv_sb[:])
                    b_i = sp // NQS
                    for j in range(4):
                        kt_i = (sp % NQS) * 4 + j
                        vps = vt_ps.tile([P, P], F32, tag="vtp")
                        nc.tensor.transpose(vps[:], vt_stage[:, j * P:(j + 1) * P], id_sb[:])
                        for hh in range(2):
                            nc.scalar.activation(
                                vaug[:, hh, b_i, kt_i, 0:DH],
                                vps[:, hh * DH:(hh + 1) * DH], Copy)

            # ---------------- Phase 2: attention per (head, batch) ----------------
            with tc.tile_pool(name="stage", bufs=4) as stage, \
                 tc.tile_pool(name="expp", bufs=4) as expp, \
                 tc.tile_pool(name="mwork", bufs=3) as mwork, \
                 tc.tile_pool(name="nwork", bufs=3) as nwork, \
                 tc.tile_pool(name="score_ps", bufs=3, space="PSUM") as score_ps, \
                 tc.tile_pool(name="scoret_ps", bufs=2, space="PSUM") as scoret_ps, \
                 tc.tile_pool(name="ctx_ps", bufs=2, space="PSUM") as ctx_ps:
                for hh in range(2):
                    h0 = hh * DH
                    for b_i in range(B):
                        pair = hh * 2 + b_i
                        tb = b_i * S
                        q_h = qt_sb[h0:h0 + DH, tb:tb + S]
                        k_h = kt_sb[h0:h0 + DH, tb:tb + S]
                        # A: full raw scores [q, k]
                        for qt_i in range(NKT):
                            for ks in range(NQS):
                                ps = score_ps.tile([P, 512], F32, tag="sc")
                                nc.tensor.matmul(
                                    ps[:], q_h[:, qt_i * P:(qt_i + 1) * P],
                                    k_h[:, ks * 512:(ks + 1) * 512],
                                    start=True, stop=True)
                                st = stage.tile([P, 512], F32, tag="st")
                                nc.vector.tensor_copy(st[:], ps[:])
                                nc.sync.dma_start(
                                    out=raw4[pair, qt_i * P:(qt_i + 1) * P,
                                             ks * 512:(ks + 1) * 512],
                                    in_=st[:])
                        # B: score^T lower tri -> exp -> @V (with ones row)
                        for qs in range(NQS):
                            cps = ctx_ps.tile([DH + 1, 512], F32, tag="ctx")
                            last = 4 * qs + 3
                            for kt_i in range(last + 1):
                                pst = scoret_ps.tile([P, 512], F32, tag="sct")
                                nc.tensor.matmul(
                                    pst[:], k_h[:, kt_i * P:(kt_i + 1) * P],
                                    q_h[:, qs * 512:(qs + 1) * 512],
                                    start=True, stop=True)
                                if kt_i >= 4 * qs:
                                    msk = mwork.tile([P, 512], F32, tag="msk")
                                    nc.vector.tensor_add(
                                        msk[:], pst[:], mneg_sb[:, kt_i - 4 * qs, :])
                                    src = msk
                                else:
                                    src = pst
                                ex = expp.tile([P, 512], F32R, tag="ex")
                                nc.scalar.activation(ex[:], src[:], Exp)
                                nc.tensor.matmul(cps[:], vaug[:, hh, b_i, kt_i, :], ex[:],
                                                 start=(kt_i == 0), stop=(kt_i == last))
                            rec = nwork.tile([1, 512], F32, tag="rec")
                            nc.vector.reciprocal(rec[:], cps[DH:DH + 1, :])
                            bc = nwork.tile([DH, 512], F32, tag="bc")
                            nc.gpsimd.partition_broadcast(bc[:], rec[:])
                            nc.vector.tensor_mul(
                                ctxT[h0:h0 + DH, tb + qs * 512:tb + (qs + 1) * 512],
                                cps[0:DH, :], bc[:])

            # ---------------- Phase 3: output projection (partial) ----------------
            with tc.tile_pool(name="ostage", bufs=4) as ostage, \
                 tc.tile_pool(name="out_ps", bufs=3, space="PSUM") as out_ps:
                for tt in range(T // P):
                    for sp in range(2):
                        ps = out_ps.tile([P, 512], F32, tag="o")
                        nc.tensor.matmul(ps[:], ctxT[:, tt * P:(tt + 1) * P],
                                         wo_sb[:, sp * 512:(sp + 1) * 512],
                                         start=True, stop=True)
                        st = ostage.tile([P, 512], F32, tag="ost")
                        nc.vector.tensor_copy(st[:], ps[:])
                        nc.sync.dma_start(
                            out=pout[tt * P:(tt + 1) * P, sp * 512:(sp + 1) * 512],
                            in_=st[:])
    nc.compile()
    return nc


def _get_compiled():
    global _COMPILED
    if _COMPILED is None:
        _COMPILED = _build()
    return _COMPILED


def _host_consts():
    mneg = np.zeros((P, 4, 512), dtype=np.float32)
    for off in range(4):
        for kk in range(P):
            mneg[kk, off, :min(off * P + kk, 512)] = -30000.0
    ones = np.ones((P, 2, B, NKT, 1), dtype=np.float32)
    ident = np.eye(P, dtype=np.float32)
    return mneg, ones, ident


def kernel(x, Wq, bq, Wk, bk, Wv, bv, Wo, bo):
    x = np.asarray(x, dtype=np.float32)
    Wq = np.asarray(Wq, dtype=np.float32)
    Wk = np.asarray(Wk, dtype=np.float32)
    Wv = np.asarray(Wv, dtype=np.float32)
    Wo = np.asarray(Wo, dtype=np.float32)
    bq = np.asarray(bq, dtype=np.float32)
    bk = np.asarray(bk, dtype=np.float32)
    bv = np.asarray(bv, dtype=np.float32)
    bo = np.asarray(bo, dtype=np.float32)

    nc = _get_compiled()
    scale = 1.0 / np.float32(np.sqrt(DH))
    xT = np.ascontiguousarray(x.reshape(T, D).T)
    mneg, ones, ident = _host_consts()

    core_ids = list(range(8))
    in_maps = []
    for c in core_ids:
        cols = slice(c * P, (c + 1) * P)
        in_maps.append({
            "xT": xT,
            "wq": np.ascontiguousarray(Wq[:, cols]) * scale,
            "wk": np.ascontiguousarray(Wk[:, cols]),
            "wv": np.ascontiguousarray(Wv[:, cols]),
            "wo": np.ascontiguousarray(Wo[cols, :]),
            "bq": (bq[cols] * scale).reshape(P, 1),
            "bk": bk[cols].reshape(P, 1),
            "bv": bv[cols].reshape(P, 1),
            "mneg": mneg,
            "ones": ones,
            "ident": ident,
        })

    trace = bool(int(os.environ.get("KERNEL_TRACE", "0")))
    if trace:
        _install_profhook()
    res = run_bass_kernel_spmd(nc, in_maps, core_ids, trace=trace)
    if trace and res.exec_time_ns is not None:
        print(f"HW exec time: {res.exec_time_ns} ns")
        kernel.last_exec_time_ns = res.exec_time_ns
        kernel.last_trace = (res.instructions_and_trace[1]
                             if res.instructions_and_trace else None)
        kernel.last_profile_json = res.profile_json

    raw = np.concatenate([res.results[c]["raw4"] for c in core_ids], axis=0)
    out = res.results[0]["pout"].copy()
    for c in core_ids[1:]:
        out += res.results[c]["pout"]
    out += bo
    return out.reshape(B, S, D), raw


def _install_profhook():
    """run_bass_kernel_spmd(trace=True) under axon needs antenv.axon_hooks,
    absent from this image; recreate the NTFF hook via ctypes."""
    import contextlib
    import ctypes
    import types

    if "antenv.axon_hooks" in sys.modules:
        return
    so_path = "/opt/axon/libaxon_pjrt.so"
    try:
        lib = ctypes.CDLL(so_path)
    except OSError:
        return
    if not hasattr(lib, "axon_start_nrt_profile"):
        return
    lib.axon_start_nrt_profile.argtypes = [ctypes.POINTER(ctypes.c_int64),
                                           ctypes.c_size_t]
    lib.axon_start_nrt_profile.restype = ctypes.c_int64
    lib.axon_stop_nrt_profile.argtypes = [ctypes.c_char_p]
    lib.axon_stop_nrt_profile.restype = ctypes.c_int64

    @contextlib.contextmanager
    def _hook(output_dir, device_ids):
        import jax
        jax.devices()
        if device_ids:
            ids = (ctypes.c_int64 * len(device_ids))(*device_ids)
            rc = lib.axon_start_nrt_profile(ids, len(device_ids))
        else:
            rc = lib.axon_start_nrt_profile(None, 0)
        if rc != 0:
            raise RuntimeError(f"axon_start_nrt_profile rc={rc}")
        try:
            yield
        finally:
            lib.axon_stop_nrt_profile(str(output_dir).encode())

    mod = types.ModuleType("antenv.axon_hooks")
    mod.get_axon_ntff_profile_hook = lambda: _hook
    sys.modules["antenv.axon_hooks"] = mod

    import concourse.bass_utils as bu
    bu.upload_artifacts = lambda tmpdir: f"file://{tmpdir}"


# revision 6
# speedup vs baseline: 1.2309x; 1.2309x over previous
"""Multi-head attention (B=2, S=2048, D=1024, H=16, d_head=64) on 8 NeuronCores.

Tensor-parallel over heads: core c owns heads {2c, 2c+1} for both batches
(pair index = 2*hh + b). Per core:
  - Q^T/K^T projections (fp32r, weights pre-rounded host-side so plain HWDGE
    DMAs suffice), V via PE transpose into Vaug tiles that carry a fused ones
    column (softmax denominators fall out of the attn@V matmul for free).
  - Scores are computed ONCE, in transposed [k, q] orientation; raw scores
    are DMA'd out as [pair, k, q] and transposed to [q, k] on the host.
    Both heads run concurrently on the PE array (row strips 0-63 / 64-127)
    since d_head=64 only fills half the 128-row contraction.
  - Causal mask is additive (-30000) applied on GpSimd from the SBUF staging
    copy; exp on ScalarE (narrowed to the unmasked column range).
  - Partial output projection (row slice of Wo); host sums the 8 partials.
Outputs per core: rawT4 [4, S, S] ([k, q] orientation) and pout [B*S, D].
"""
import os
import sys

sys.path.insert(0, "/opt/trn_rl_repo")

import numpy as np

import concourse.bacc as bacc
import concourse.mybir as mybir
from concourse.tile import TileContext
from concourse.bass_utils import run_bass_kernel_spmd

P = 128
B, S, D, H, DH = 2, 2048, 1024, 16, 64
T = B * S                  # 4096 tokens
NDT = D // P               # 8 d-model tiles
NSP = T // 512             # 8 token spans (4 per batch)
NKT = S // P               # 16 key tiles per batch
NQS = S // 512             # 4 q spans per batch
F32 = mybir.dt.float32
F32R = mybir.dt.float32r
Copy = mybir.ActivationFunctionType.Copy
Identity = mybir.ActivationFunctionType.Identity
Exp = mybir.ActivationFunctionType.Exp

_COMPILED = None


def _build():
    nc = bacc.Bacc("TRN2", target_bir_lowering=False, debug=False, num_devices=8)

    xT = nc.declare_dram_parameter("xT", [D, T], F32R, isOutput=False)
    wq = nc.declare_dram_parameter("wq", [D, P], F32R, isOutput=False)
    wk = nc.declare_dram_parameter("wk", [D, P], F32R, isOutput=False)
    wv = nc.declare_dram_parameter("wv", [D, P], F32R, isOutput=False)
    wo = nc.declare_dram_parameter("wo", [P, D], F32R, isOutput=False)
    bq = nc.declare_dram_parameter("bq", [P, 1], F32, isOutput=False)
    bk = nc.declare_dram_parameter("bk", [P, 1], F32, isOutput=False)
    bv = nc.declare_dram_parameter("bv", [P, 1], F32, isOutput=False)
    mneg = nc.declare_dram_parameter("mneg", [P, 4, 512], F32, isOutput=False)
    ones = nc.declare_dram_parameter("ones", [P, 2, B, NKT, 1], F32R, isOutput=False)
    ident = nc.declare_dram_parameter("ident", [P, P], F32, isOutput=False)

    rawT4 = nc.declare_dram_parameter("rawT4", [4, S, S], F32, isOutput=True)
    pout = nc.declare_dram_parameter("pout", [T, D], F32, isOutput=True)

    with TileContext(nc) as tc:
        with tc.tile_pool(name="persist", bufs=1) as persist, \
             tc.tile_pool(name="consts", bufs=1) as consts, \
             tc.tile_pool(name="xsp", bufs=2) as xsp, \
             tc.tile_pool(name="vstage", bufs=2) as vstage, \
             tc.tile_pool(name="stage", bufs=4) as stage, \
             tc.tile_pool(name="expp", bufs=6) as expp, \
             tc.tile_pool(name="mwork", bufs=3) as mwork, \
             tc.tile_pool(name="nwork", bufs=4) as nwork, \
             tc.tile_pool(name="ostage", bufs=3) as ostage, \
             tc.tile_pool(name="mm_ps", bufs=5, space="PSUM") as mm_ps, \
             tc.tile_pool(name="vt_ps", bufs=1, space="PSUM") as vt_ps, \
             tc.tile_pool(name="ctx_ps", bufs=2, space="PSUM") as ctx_ps:
            qt_sb = persist.tile([P, T], F32R, tag="qt")
            kt_sb = persist.tile([P, T], F32R, tag="kt")
            ctxT = persist.tile([P, T], F32R, tag="ctxT")
            vaug = persist.tile([P, 2, B, NKT, DH + 1], F32R, tag="vaug")
            wq_sb = consts.tile([P, NDT, P], F32R, tag="wq")
            wk_sb = consts.tile([P, NDT, P], F32R, tag="wk")
            wv_sb = consts.tile([P, NDT, P], F32R, tag="wv")
            wo_sb = consts.tile([P, D], F32R, tag="wo")
            bq_sb = consts.tile([P, 1], F32, tag="bq")
            bk_sb = consts.tile([P, 1], F32, tag="bk")
            bv_sb = consts.tile([P, 1], F32, tag="bv")
            mneg_sb = consts.tile([P, 4, 512], F32, tag="mneg")
            id_sb = consts.tile([P, P], F32, tag="ident")

            for w_dram, w_tile in ((wq, wq_sb), (wk, wk_sb), (wv, wv_sb)):
                nc.sync.dma_start(
                    out=w_tile[:], in_=w_dram[:].rearrange("(dt p) m -> p dt m", p=P))
            nc.sync.dma_start(out=wo_sb[:], in_=wo[:])
            nc.sync.dma_start(out=vaug[:, :, :, :, DH:DH + 1], in_=ones[:])
            nc.sync.dma_start(out=bq_sb[:], in_=bq[:])
            nc.sync.dma_start(out=bk_sb[:], in_=bk[:])
            nc.sync.dma_start(out=bv_sb[:], in_=bv[:])
            nc.sync.dma_start(out=mneg_sb[:], in_=mneg[:])
            nc.sync.dma_start(out=id_sb[:], in_=ident[:])

            xT_v = xT[:].rearrange("(dt p) t -> p dt t", p=P)

            evac_i = 0

            def phase1_span(sp):
                t0 = sp * 512
                xs = xsp.tile([P, NDT, 512], F32R, tag="xs")
                nc.sync.dma_start(out=xs[:], in_=xT_v[:, :, t0:t0 + 512])
                for w_tile, b_tile, dst in (
                        (wq_sb, bq_sb, qt_sb), (wk_sb, bk_sb, kt_sb)):
                    ps = mm_ps.tile([P, 512], F32, tag="mm")
                    for dt_i in range(NDT):
                        nc.tensor.matmul(ps[:], w_tile[:, dt_i, :], xs[:, dt_i, :],
                                         start=(dt_i == 0), stop=(dt_i == NDT - 1))
                    nc.scalar.activation(dst[:, t0:t0 + 512], ps[:], Identity,
                                         bias=b_tile[:])
                ps = mm_ps.tile([P, 512], F32, tag="mm")
                for dt_i in range(NDT):
                    nc.tensor.matmul(ps[:], wv_sb[:, dt_i, :], xs[:, dt_i, :],
                                     start=(dt_i == 0), stop=(dt_i == NDT - 1))
                vt_stage = vstage.tile([P, 512], F32, tag="vt")
                nc.scalar.activation(vt_stage[:], ps[:], Identity, bias=bv_sb[:])
                b_i = sp // NQS
                for j in range(4):
                    kt_i = (sp % NQS) * 4 + j
                    vps = vt_ps.tile([P, P], F32, tag="vtp")
                    nc.tensor.transpose(vps[:], vt_stage[:, j * P:(j + 1) * P], id_sb[:])
                    for hh in range(2):
                        nc.scalar.activation(
                            vaug[:, hh, b_i, kt_i, 0:DH],
                            vps[:, hh * DH:(hh + 1) * DH], Copy)

            def phase2_batch(b_i):
                nonlocal evac_i
                tb = b_i * S
                for qs in range(NQS):
                    cps = []
                    for _h in range(2):
                        cps_t = ctx_ps.tile([DH + 1, 512], F32, tag="ctx")
                        cps.append(cps_t)
                    last_kt = 4 * qs + 3
                    for kb in range(4):
                        stg = []
                        for _h in range(2):
                            stg_t = stage.tile([P, 4, 512], F32, tag="st")
                            stg.append(stg_t)
                        for j in range(4):
                            kt_i = kb * 4 + j
                            psts = []
                            for hh in range(2):
                                h0 = hh * DH
                                pst = mm_ps.tile([P, 512], F32, tag="mm")
                                nc.tensor.matmul(
                                    pst[:],
                                    kt_sb[h0:h0 + DH, tb + kt_i * P:tb + (kt_i + 1) * P],
                                    qt_sb[h0:h0 + DH, tb + qs * 512:tb + (qs + 1) * 512],
                                    start=True, stop=True)
                                psts.append(pst)
                            for hh in range(2):
                                # evacuate raw score tile to staging (alternate engines)
                                dst = stg[hh][:, j, :]
                                if evac_i % 3 == 0:
                                    nc.scalar.activation(dst, psts[hh][:], Copy)
                                else:
                                    nc.vector.tensor_copy(dst, psts[hh][:])
                                evac_i += 1
                            if kt_i <= last_kt:
                                off = kt_i - 4 * qs
                                c0 = max(off, 0) * P  # first unmasked column
                                for hh in range(2):
                                    if off >= 0:  # diagonal tile: apply mask in SBUF
                                        msk = mwork.tile([P, 512], F32, tag="msk")
                                        nc.gpsimd.tensor_add(
                                            msk[:, c0:], stg[hh][:, j, c0:],
                                            mneg_sb[:, off, c0:])
                                        src, sc0 = msk, c0
                                    else:
                                        src, sc0 = stg[hh], None
                                    ex = expp.tile([P, 512], F32R, tag="ex")
                                    if sc0 is not None:
                                        nc.scalar.activation(ex[:, c0:], src[:, c0:], Exp)
                                    else:
                                        nc.scalar.activation(ex[:, :], src[:, j, :], Exp)
                                    nc.tensor.matmul(
                                        cps[hh][:, c0:], vaug[:, hh, b_i, kt_i, :],
                                        ex[:, c0:],
                                        start=(kt_i == 0), stop=(kt_i == last_kt))
                        for hh in range(2):
                            pair = hh * 2 + b_i
                            nc.sync.dma_start(
                                out=rawT4[pair, kb * 512:(kb + 1) * 512,
                                          qs * 512:(qs + 1) * 512].rearrange(
                                              "(kt p) q -> p kt q", p=P),
                                in_=stg[hh][:])
                    for hh in range(2):
                        h0 = hh * DH
                        rec = nwork.tile([1, 512], F32, tag="rec")
                        nc.vector.reciprocal(rec[:], cps[hh][DH:DH + 1, :])
                        bc = nwork.tile([DH, 512], F32, tag="bc")
                        nc.gpsimd.partition_broadcast(bc[:], rec[:])
                        nc.vector.tensor_mul(
                            ctxT[h0:h0 + DH, tb + qs * 512:tb + (qs + 1) * 512],
                            cps[hh][0:DH, :], bc[:])

            def phase3_ttile(tt):
                ost = ostage.tile([P, 2, 512], F32, tag="ost")
                for sp in range(2):
                    ps = mm_ps.tile([P, 512], F32, tag="mm")
                    nc.tensor.matmul(ps[:], ctxT[:, tt * P:(tt + 1) * P],
                                     wo_sb[:, sp * 512:(sp + 1) * 512],
                                     start=True, stop=True)
                    nc.vector.tensor_copy(ost[:, sp, :], ps[:])
                nc.sync.dma_start(out=pout[tt * P:(tt + 1) * P, :],
                                  in_=ost[:].rearrange("p s q -> p (s q)"))

            for sp in range(4):
                phase1_span(sp)
            for sp in range(4, 8):
                phase1_span(sp)
            phase2_batch(0)
            phase2_batch(1)
            for tt in range(T // P):
                phase3_ttile(tt)
    nc.compile()
    return nc


def _get_compiled():
    global _COMPILED
    if _COMPILED is None:
        _COMPILED = _build()
    return _COMPILED


def _rne12(x):
    """Round fp32 to the fp32r grid (round-to-nearest-even, drop 12 mantissa
    bits) — matches the hardware's fp32->fp32r cast bit-exactly."""
    xi = np.ascontiguousarray(x, dtype=np.float32).view(np.uint32).astype(np.uint64)
    bias = ((xi >> 12) & 1) + 0x7FF
    r = ((xi + bias) >> 12) << 12
    return r.astype(np.uint32).view(np.float32).reshape(np.shape(x))


def _host_consts():
    mneg = np.zeros((P, 4, 512), dtype=np.float32)
    for off in range(4):
        for kk in range(P):
            mneg[kk, off, :min(off * P + kk, 512)] = -30000.0
    ones = np.ones((P, 2, B, NKT, 1), dtype=np.float32)
    ident = np.eye(P, dtype=np.float32)
    return mneg, ones, ident


def kernel(x, Wq, bq, Wk, bk, Wv, bv, Wo, bo):
    x = np.asarray(x, dtype=np.float32)
    Wq = np.asarray(Wq, dtype=np.float32)
    Wk = np.asarray(Wk, dtype=np.float32)
    Wv = np.asarray(Wv, dtype=np.float32)
    Wo = np.asarray(Wo, dtype=np.float32)
    bq = np.asarray(bq, dtype=np.float32)
    bk = np.asarray(bk, dtype=np.float32)
    bv = np.asarray(bv, dtype=np.float32)
    bo = np.asarray(bo, dtype=np.float32)

    nc = _get_compiled()
    scale = np.float32(1.0 / np.sqrt(DH))
    xTr = _rne12(np.ascontiguousarray(x.reshape(T, D).T))
    mneg, ones, ident = _host_consts()

    core_ids = list(range(8))
    in_maps = []
    for c in core_ids:
        cols = slice(c * P, (c + 1) * P)
        in_maps.append({
            "xT": xTr,
            "wq": _rne12(Wq[:, cols] * scale),
            "wk": _rne12(Wk[:, cols]),
            "wv": _rne12(Wv[:, cols]),
            "wo": _rne12(Wo[cols, :]),
            "bq": (bq[cols] * scale).reshape(P, 1),
            "bk": bk[cols].reshape(P, 1),
            "bv": bv[cols].reshape(P, 1),
            "mneg": mneg,
            "ones": ones,
            "ident": ident,
        })

    trace = bool(int(os.environ.get("KERNEL_TRACE", "0")))
    if trace:
        _install_profhook()
    res = run_bass_kernel_spmd(nc, in_maps, core_ids, trace=trace)
    if trace and res.exec_time_ns is not None:
        print(f"HW exec time: {res.exec_time_ns} ns")
        kernel.last_exec_time_ns = res.exec_time_ns
        kernel.last_trace = (res.instructions_and_trace[1]
                             if res.instructions_and_trace else None)
        kernel.last_profile_json = res.profile_json

    raw = np.empty((4 * 8, S, S), dtype=np.float32)
    for c in core_ids:
        rt = res.results[c]["rawT4"]
        for i in range(4):
            raw[4 * c + i] = rt[i].T
    out = res.results[0]["pout"].copy()
    for c in core_ids[1:]:
        out += res.results[c]["pout"]
    out += bo
    return out.reshape(B, S, D), raw


def _install_profhook():
    """run_bass_kernel_spmd(trace=True) under axon needs antenv.axon_hooks,
    absent from this image; recreate the NTFF hook via ctypes."""
    import contextlib
    import ctypes
    import types

    if "antenv.axon_hooks" in sys.modules:
        return
    so_path = "/opt/axon/libaxon_pjrt.so"
    try:
        lib = ctypes.CDLL(so_path)
    except OSError:
        return
    if not hasattr(lib, "axon_start_nrt_profile"):
        return
    lib.axon_start_nrt_profile.argtypes = [ctypes.POINTER(ctypes.c_int64),
                                           ctypes.c_size_t]
    lib.axon_start_nrt_profile.restype = ctypes.c_int64
    lib.axon_stop_nrt_profile.argtypes = [ctypes.c_char_p]
    lib.axon_stop_nrt_profile.restype = ctypes.c_int64

    @contextlib.contextmanager
    def _hook(output_dir, device_ids):
        import jax
        jax.devices()
        if device_ids:
            ids = (ctypes.c_int64 * len(device_ids))(*device_ids)
            rc = lib.axon_start_nrt_profile(ids, len(device_ids))
        else:
            rc = lib.axon_start_nrt_profile(None, 0)
        if rc != 0:
            raise RuntimeError(f"axon_start_nrt_profile rc={rc}")
        try:
            yield
        finally:
            lib.axon_stop_nrt_profile(str(output_dir).encode())

    mod = types.ModuleType("antenv.axon_hooks")
    mod.get_axon_ntff_profile_hook = lambda: _hook
    sys.modules["antenv.axon_hooks"] = mod

    import concourse.bass_utils as bu
    bu.upload_artifacts = lambda tmpdir: f"file://{tmpdir}"
